# revision 2
# baseline (speedup 1.0000x reference)
"""CRF log-likelihood kernel for Trainium2 (Bass/Tile), 8-core data parallel.

Algorithm (per core, BC=32 sequences):
  Forward algorithm runs in the exp domain: one small matmul per timestep with
  stationary weights W = exp(transitions)^T, plus one DVE tensor_tensor
  multiply by precomputed per-step emission factors.

  Emissions are pre-normalized so no mid-scan rescaling is needed:
      E[i,b,t] = exp(x[b,t,i]) / s_hat[b,t] * exp(-kappa) * m[b,t],  i<126
      E[127,b,t] = 1 - m[b,t]          (the "park in STOP" trick)
      E[126,b,t] = 0
  with s_hat = sum_i exp(x), kappa = ln(sum exp(trans)[:126,:126]/126).
  The log normalizers are accumulated into C[b] = sum_t m*(ln s_hat + kappa).

  Setting M[STOP,STOP]=1 makes a finished sequence's state a frozen one-hot at
  STOP holding exp(partition - C): masking and the final STOP transition both
  disappear into the uniform per-step dynamics. One extra step at t=L captures
  sequences with x_len == L.  partition = ln A_final[127] + C.

  Gold score: per-(b,t) one-hot matrices contracted on PE into per-sequence
  transition-count matrices K_b (pair term = <trans, K_b>), fused
  multiply-reduce for the emission gather, and mask-difference matmuls for the
  final-tag STOP lookup.
"""

import os

import numpy as np

import concourse.bass as bass
import concourse.bacc as bacc
import concourse.mybir as mybir
import concourse.tile as tile
from concourse.bass_utils import run_bass_kernel_spmd

F32 = mybir.dt.float32
BF16 = mybir.dt.bfloat16
I32 = mybir.dt.int32
AX = mybir.AxisListType
OP = mybir.AluOpType
ACT = mybir.ActivationFunctionType

B_FULL, L_FULL, D = 256, 1024, 126
T = 128
START, STOP = 126, 127
N_CORES = 8


def build_nc(L=L_FULL, BC=B_FULL // N_CORES):
    NCH = L // 128
    assert L % 128 == 0 and 4 * BC <= 128
    nc = bacc.Bacc(None)

    x_d = nc.dram_tensor("x", [BC, L, D], F32, kind="ExternalInput")
    tr_d = nc.dram_tensor("transitions", [T, T], F32, kind="ExternalInput")
    mask_d = nc.dram_tensor("x_mask", [BC, L], F32, kind="ExternalInput")
    maskn_d = nc.dram_tensor("x_mask_next", [BC, L], F32, kind="ExternalInput")
    tag_d = nc.dram_tensor("true_tag", [BC, L], I32, kind="ExternalInput")
    tagp_d = nc.dram_tensor("tag_prev", [BC, L], I32, kind="ExternalInput")
    out_d = nc.dram_tensor("out", [BC], F32, kind="ExternalOutput")

    with tile.TileContext(nc) as tc:
        with (
            tc.tile_pool(name="const", bufs=1) as cpool,
            tc.tile_pool(name="work", bufs=2) as wpool,
            tc.tile_pool(name="er", bufs=1) as erpool,
            tc.tile_pool(name="scan", bufs=3) as apool,
            tc.tile_pool(name="psP", bufs=2, space="PSUM") as psP,
            tc.tile_pool(name="psT", bufs=2, space="PSUM") as psT,
            tc.tile_pool(name="psK", bufs=1, space="PSUM") as psK,
            tc.tile_pool(name="psA", bufs=1, space="PSUM") as psA,
            tc.tile_pool(name="psM", bufs=1, space="PSUM") as psM,
        ):
            # ---------------- setup: constants ----------------
            trans_sb = cpool.tile([T, T], F32)
            nc.sync.dma_start(trans_sb[:], tr_d[:])
            transT_sb = cpool.tile([T, T], F32)
            nc.sync.dma_start(transT_sb[:], tr_d[:].rearrange("i j -> j i"))

            # host pre-sets transitions[127,127]=0 so exp gives M[STOP,STOP]=1
            W_sb = cpool.tile([T, T], F32)  # W[j,i] = exp(trans[i,j])
            nc.scalar.activation(W_sb[:], transT_sb[:], ACT.Exp)

            iota_i = cpool.tile([T, T], I32)
            nc.gpsimd.iota(iota_i[:], pattern=[[1, T]], base=0, channel_multiplier=0)
            iota_f = cpool.tile([T, T], F32)
            nc.vector.tensor_copy(iota_f[:], iota_i[:])
            iota_ci = cpool.tile([T, 1], I32)
            nc.gpsimd.iota(iota_ci[:], pattern=[[1, 1]], base=0, channel_multiplier=1)
            iota_cf = cpool.tile([T, 1], F32)
            nc.vector.tensor_copy(iota_cf[:], iota_ci[:])
            ident = cpool.tile([T, T], F32)
            nc.vector.tensor_scalar(ident[:], iota_f[:], iota_cf[:], None, OP.is_equal)

            stopcol = cpool.tile([T, 1], F32)
            nc.vector.tensor_scalar(stopcol[:], iota_cf[:], float(STOP), None, OP.is_equal)
            startcol = cpool.tile([T, 1], F32)
            nc.vector.tensor_scalar(startcol[:], iota_cf[:], float(START), None, OP.is_equal)

            ones_col = cpool.tile([T, 1], F32)
            nc.vector.memset(ones_col[:], 1.0)
            ones_row = cpool.tile([1, T], F32)
            nc.vector.memset(ones_row[:], 1.0)

            # kappa: S = sum(exp(trans)[0:126,0:126]); g = S/126, ig = 126/S
            s1p = psM.tile([D, 1], F32, tag="misc")
            nc.tensor.matmul(s1p[:], W_sb[0:D, 0:D], ones_col[0:D, :])
            s1c = cpool.tile([D, 1], F32)
            nc.scalar.copy(s1c[:], s1p[:])
            Sp = psM.tile([1, 1], F32, tag="misc")
            nc.tensor.matmul(Sp[:], s1c[:], ones_col[0:D, :])
            g11 = cpool.tile([1, 1], F32)
            nc.scalar.mul(g11[:], Sp[:], 1.0 / (D * D))
            ig11 = cpool.tile([1, 1], F32)
            nc.vector.reciprocal(ig11[:], g11[:])

            gcp = psM.tile([T, 1], F32, tag="misc")
            nc.tensor.matmul(gcp[:], ones_row[:], g11[:])
            g_col = cpool.tile([T, 1], F32)
            nc.scalar.copy(g_col[:], gcp[:])
            igp = psM.tile([T, 1], F32, tag="misc")
            nc.tensor.matmul(igp[:], ones_row[:], ig11[:])
            ig_col = cpool.tile([T, 1], F32)
            nc.scalar.copy(ig_col[:], igp[:])

            # ---------------- big persistent buffers ----------------
            E_buf = cpool.tile([T, BC, L + 1], F32)
            # extra stop-capture step at t=L: one-hot(STOP) for every sequence
            for b in range(BC):
                nc.vector.tensor_copy(E_buf[:, b, L : L + 1], stopcol[:])

            # accumulators: EM | PAIR | TL at columns [0:BC][BC:2BC][2BC:3BC]
            # (each written by standalone start+stop matmuls: TRN2 start=True
            #  zeroes the whole 2KB bank region, so no interleaved accum groups)
            ACC3 = psA.tile([1, 3 * BC], F32, tag="acc3")
            crun = cpool.tile([1, BC], F32, name="crun0")
            nc.vector.memset(crun[:], 0.0)
            crun_box = [crun]

            # ---------------- phase functions ----------------
            def pass_a(c):
                t0 = c * 128
                xt = wpool.tile([128, BC, D], F32, tag="xt")
                nc.sync.dma_start(
                    xt[:], x_d[:, t0 : t0 + 128, :].rearrange("b t d -> t b d")
                )
                m_cols = wpool.tile([128, BC], F32, tag="mc")
                nc.sync.dma_start(
                    m_cols[:], mask_d[:, t0 : t0 + 128].rearrange("b t -> t b")
                )
                er = erpool.tile([128, BC, T], F32, tag="er")
                nc.vector.memset(er[:, :, D], 0.0)  # col 126 (START row after T)
                sh = wpool.tile([128, BC], F32, tag="sh")
                for b in range(BC):
                    nc.scalar.activation(
                        er[:, b, 0:D], xt[:, b, :], ACT.Exp,
                        accum_out=sh[:, b : b + 1],
                    )
                rsh = wpool.tile([128, BC], F32, tag="rsh")
                nc.vector.reciprocal(rsh[:], sh[:])
                rsm = wpool.tile([128, BC], F32, tag="rsm")
                nc.vector.scalar_tensor_tensor(
                    rsm[:], rsh[:], ig_col[:], m_cols[:], OP.mult, OP.mult
                )
                lnsh = wpool.tile([128, BC], F32, tag="lnsh")
                nc.scalar.activation(lnsh[:], sh[:], ACT.Ln, scale=g_col[:])
                # C partial for this chunk: per-b masked column dots
                Cp = psA.tile([1, BC], F32, tag="cp", bufs=1)
                for b in range(BC):
                    nc.tensor.matmul(
                        Cp[:, b : b + 1],
                        lnsh[:, b : b + 1],
                        m_cols[:, b : b + 1],
                        skip_group_check=True,
                    )
                crun_new = wpool.tile([1, BC], F32, tag="crun")
                nc.vector.tensor_add(crun_new[:], crun_box[0][:], Cp[:])
                crun_box[0] = crun_new
                for b in range(BC):
                    # scale emission block, fill done-flag col, transpose out
                    nc.gpsimd.tensor_scalar(
                        er[:, b, 0:D], er[:, b, 0:D], rsm[:, b : b + 1], None, OP.mult
                    )
                    nc.vector.tensor_scalar(
                        er[:, b, STOP : STOP + 1],
                        m_cols[:, b : b + 1],
                        -1.0,
                        1.0,
                        OP.mult,
                        OP.add,
                    )
                    etp = psT.tile([T, 128], F32, tag="etp")
                    nc.tensor.transpose(etp[:], er[:, b, :], ident[:])
                    nc.scalar.copy(E_buf[:, b, t0 : t0 + 128], etp[:])

            def scan_steps(ts, te, a_prev):
                for t in range(ts, te):
                    P = psP.tile([T, BC], F32, tag="P")
                    nc.tensor.matmul(P[:], W_sb[:], a_prev[:])
                    if t < L:
                        a_new = apool.tile([T, BC], F32, tag="A")
                    else:
                        a_new = cpool.tile([T, BC], F32, name="A_last")
                    nc.vector.tensor_mul(a_new[:], P[:], E_buf[:, :, t])
                    a_prev = a_new
                return a_prev

            def pass_b(b):
                xb = wpool.tile([128, NCH, D], F32, tag="xb")
                nc.sync.dma_start(
                    xb[:], x_d[b, :, :].rearrange("(c p) d -> p c d", p=128)
                )
                tag_i = wpool.tile([128, NCH], I32, tag="tgi")
                nc.sync.dma_start(
                    tag_i[:], tag_d[b, :].rearrange("(c p) -> p c", p=128)
                )
                tagf = wpool.tile([128, NCH], F32, tag="tgf")
                nc.vector.tensor_copy(tagf[:], tag_i[:])
                # shifted tags (tag_{t-1}); row0 col0 <- START after convert
                tagp_i = wpool.tile([128, NCH], I32, tag="tpi")
                nc.sync.dma_start(
                    tagp_i[:], tagp_d[b, :].rearrange("(c p) -> p c", p=128)
                )
                tagpf = wpool.tile([128, NCH], F32, tag="tpf")
                nc.vector.tensor_copy(tagpf[:], tagp_i[:])

                mb = wpool.tile([128, NCH], F32, tag="mb")
                nc.sync.dma_start(mb[:], mask_d[b, :].rearrange("(c p) -> p c", p=128))
                # m shifted by +1 (m_{t+1}), last element 0
                mb2 = wpool.tile([128, NCH], F32, tag="mb2")
                nc.sync.dma_start(
                    mb2[:], maskn_d[b, :].rearrange("(c p) -> p c", p=128)
                )
                dd = wpool.tile([128, NCH], F32, tag="dd")
                nc.vector.tensor_sub(dd[:], mb[:], mb2[:])

                PB = os.environ.get("KERNEL_PB", "12345")
                Kp = psK.tile([D, D + 1], F32, tag="K")
                emc = wpool.tile([128, NCH], F32, tag="emc")
                if "2" not in PB:
                    nc.vector.memset(emc[:], 0.0)
                for c in range(NCH):
                    if "2" not in PB:
                        break
                    Hcm = wpool.tile([128, D + 1], BF16, tag="Hcm")
                    nc.vector.tensor_scalar(
                        Hcm[:],
                        iota_f[:, 0 : D + 1],
                        tagf[:, c : c + 1],
                        mb[:, c : c + 1],
                        OP.is_equal,
                        OP.mult,
                    )
                    Hp = wpool.tile([128, D + 1], BF16, tag="Hp")
                    nc.gpsimd.tensor_scalar(
                        Hp[:],
                        iota_f[:, 0 : D + 1],
                        tagpf[:, c : c + 1],
                        None,
                        OP.is_equal,
                    )
                    if "3" in PB:
                        nc.tensor.matmul(
                            Kp[:],
                            Hcm[:, 0:D],
                            Hp[:],
                            start=(c == 0),
                            stop=(c == NCH - 1),
                        )
                    # emission gather: sum_i x*Hcm (per-t values into emc col c)
                    if "4" in PB:
                        scr = wpool.tile([128, D], F32, tag="scr")
                        nc.vector.scalar_tensor_tensor(
                            scr[:], xb[:, c, :], 1.0, Hcm[:, 0:D],
                            OP.mult, OP.mult,
                            accum_out=emc[:, c : c + 1],
                        )
                    else:
                        nc.vector.memset(emc[:, c : c + 1], 0.0)
                if "3" not in PB or "2" not in PB:
                    nc.vector.memset(Kp[:], 0.0)
                # emission: sum over (t, c)
                emr = wpool.tile([128, 1], F32, tag="emr")
                nc.vector.tensor_reduce(emr[:], emc[:], AX.X, OP.add)
                nc.tensor.matmul(
                    ACC3[:, b : b + 1], emr[:], ones_col[:], skip_group_check=True
                )
                # tag_last: sum_t tag * (m_t - m_{t+1})
                tdd = wpool.tile([128, NCH], F32, tag="tdd")
                nc.vector.tensor_mul(tdd[:], tagf[:], dd[:])
                tdr = wpool.tile([128, 1], F32, tag="tdr")
                nc.vector.tensor_reduce(tdr[:], tdd[:], AX.X, OP.add)
                nc.tensor.matmul(
                    ACC3[:, 2 * BC + b : 2 * BC + b + 1],
                    tdr[:],
                    ones_col[:],
                    skip_group_check=True,
                )
                # retire K_b: pair = <trans, K>
                K_sb = wpool.tile([D, D + 1], F32, tag="Ksb")
                nc.scalar.copy(K_sb[:], Kp[:])
                tk = wpool.tile([D, D + 1], F32, tag="tk")
                nc.vector.tensor_mul(tk[:], K_sb[:], trans_sb[0:D, 0 : D + 1])
                tkr = wpool.tile([D, 1], F32, tag="tkr")
                nc.vector.tensor_reduce(tkr[:], tk[:], AX.X, OP.add)
                nc.tensor.matmul(
                    ACC3[:, BC + b : BC + b + 1],
                    tkr[:],
                    ones_col[0:D, :],
                    skip_group_check=True,
                )

            # ---------------- emit program ----------------
            a0 = apool.tile([T, BC], F32, tag="A")
            for b in range(BC):
                nc.vector.tensor_copy(a0[:, b : b + 1], startcol[:])

            PARTS = os.environ.get("KERNEL_PARTS", "abs")  # a=passA b=passB s=scan
            if "a" in PARTS:
                pass_a(0)
                if NCH > 1:
                    pass_a(1)
            a_cur = a0
            bs_per_ch = -(-BC // NCH)  # ceil
            for k in range(NCH):
                if "a" in PARTS and k + 2 < NCH:
                    pass_a(k + 2)
                if "s" in PARTS:
                    a_cur = scan_steps(k * 128, (k + 1) * 128, a_cur)
                if "b" in PARTS:
                    for b in range(k * bs_per_ch, min((k + 1) * bs_per_ch, BC)):
                        pass_b(b)
            if "s" in PARTS:
                a_cur = scan_steps(L, L + 1, a_cur)

            # ---------------- finale ----------------
            if "b" not in PARTS:
                nc.vector.memset(ACC3[:], 0.0)
            # partition-side: A_final[STOP, :] extracted via one-hot matmul
            lnp = psM.tile([1, BC], F32, tag="misc")
            nc.tensor.matmul(lnp[:], stopcol[:], a_cur[:])
            lnA = cpool.tile([1, BC], F32)
            if "s" in PARTS:
                nc.scalar.activation(lnA[:], lnp[:], ACT.Ln)
            else:
                nc.scalar.copy(lnA[:], lnp[:])

            # stop term: trans[STOP, tag_last] = transT[tag_last, STOP]
            tl_sb = cpool.tile([1, BC], F32)
            nc.scalar.copy(tl_sb[:], ACC3[:, 2 * BC : 3 * BC])
            tlcp2 = psM.tile([BC, 1], F32, tag="misc")
            nc.tensor.matmul(tlcp2[:], tl_sb[:], ones_row[0:1, 0:1])
            tl_col = cpool.tile([BC, 1], F32)
            nc.scalar.copy(tl_col[:], tlcp2[:])
            Hl = cpool.tile([BC, D], F32)
            nc.vector.tensor_scalar(
                Hl[:], iota_f[0:BC, 0:D], tl_col[:], None, OP.is_equal
            )
            hlp = psM.tile([D, BC], F32, tag="misc")
            nc.tensor.transpose(hlp[:], Hl[:], ident[0:BC, 0:BC])
            HlT = cpool.tile([D, BC], F32)
            nc.scalar.copy(HlT[:], hlp[:])
            stp = psM.tile([1, BC], F32, tag="misc")
            nc.tensor.matmul(stp[:], transT_sb[0:D, STOP : STOP + 1], HlT[:])
            stop_sb = cpool.tile([1, BC], F32)
            nc.scalar.copy(stop_sb[:], stp[:])

            c_sb = crun_box[0]
            em_sb = cpool.tile([1, BC], F32)
            nc.scalar.copy(em_sb[:], ACC3[:, 0:BC])
            pair_sb = cpool.tile([1, BC], F32)
            nc.scalar.copy(pair_sb[:], ACC3[:, BC : 2 * BC])

            s1 = cpool.tile([1, BC], F32)
            nc.vector.tensor_add(s1[:], pair_sb[:], em_sb[:])
            s2 = cpool.tile([1, BC], F32)
            nc.vector.tensor_add(s2[:], s1[:], stop_sb[:])
            s3 = cpool.tile([1, BC], F32)
            nc.vector.tensor_add(s3[:], lnA[:], c_sb[:])
            res = cpool.tile([1, BC], F32)
            nc.vector.tensor_sub(res[:], s2[:], s3[:])
            nc.sync.dma_start(out_d[:].rearrange("(o b) -> o b", o=1), res[:])

    nc.compile()
    return nc


_NC_CACHE = {}


def _get_nc(L, BC):
    key = (L, BC)
    if key not in _NC_CACHE:
        _NC_CACHE[key] = build_nc(L, BC)
    return _NC_CACHE[key]


def kernel(x, transitions, x_mask, x_len, true_tag):
    x = np.ascontiguousarray(np.asarray(x, dtype=np.float32))
    transitions = np.asarray(transitions, dtype=np.float32).copy()
    transitions[STOP, STOP] = 0.0  # exp -> 1: frozen STOP self-loop
    x_mask = np.ascontiguousarray(np.asarray(x_mask, dtype=np.float32))
    true_tag = np.ascontiguousarray(np.asarray(true_tag, dtype=np.int32))
    L = x.shape[1]
    mask_next = np.ascontiguousarray(
        np.concatenate([x_mask[:, 1:], np.zeros_like(x_mask[:, :1])], axis=1)
    )
    tag_prev = np.ascontiguousarray(
        np.concatenate(
            [np.full_like(true_tag[:, :1], START), true_tag[:, : L - 1]], axis=1
        )
    )

    B = x.shape[0]
    BC = B // N_CORES
    nc = _get_nc(L, BC)
    in_maps = []
    for ci in range(N_CORES):
        s = slice(ci * BC, (ci + 1) * BC)
        in_maps.append(
            {
                "x": x[s],
                "transitions": transitions,
                "x_mask": x_mask[s],
                "x_mask_next": mask_next[s],
                "true_tag": true_tag[s],
                "tag_prev": tag_prev[s],
            }
        )
    r = run_bass_kernel_spmd(nc, in_maps, core_ids=list(range(N_CORES)))
    global LAST_RESULTS
    LAST_RESULTS = r
    return np.concatenate([m["out"] for m in r.results]).astype(np.float32)

